# revision 15
# baseline (speedup 1.0000x reference)
"""Per-edge dot product kernel for Trainium2 (8 NeuronCores).

Computes out[e] = sum(h[src[e]] * h[dst[e]], axis=-1) for
h: [100000, 64] f32, src/dst: [1000000] int indices.

Bottleneck analysis (HW, bedrock image -- no extended GPSIMD ucode, so
dma_gather/ap_gather are unavailable and the only data-dependent gather
is core SWDGE indirect DMA):
  - indirect_dma_start costs ~994ns fixed SWDGE overhead + 0.34ns/desc,
    and HW supports only ONE offset per partition -> 128 rows / ~1.1us
    instruction, serialized on the GPSIMD engine.  The old kernel issued
    2*E/8/128 = 1968 such instructions per core -> 2.2ms GPSIMD time.

Design (halves the indirect-DMA count by moving the src side to PE):
  - Host sorts edges by src; core c takes the c-th contiguous 125k slice
    of sorted order, so its src values span ~12.6k contiguous node rows.
    That slice of h is the core's "table", DMA'd sequentially (no
    descriptors-per-row) and held in SBUF as bf16 [128, 104 blocks, 64].
  - Src rows are then gathered ON-CHIP via one-hot matmuls: for each
    chunk of 128 edges (all in one 128-row table block by construction),
    PE computes onehot[128nodes,128edges]^T @ table_blk[128,64] ->
    gathered rows [128 edges, 64] in PSUM.  One-hots are built by DVE
    is_equal(broadcast(srclocal), iota) -- no GPSIMD involvement.
  - Dst rows (random order) still use indirect DMA: 8 gathers per
    1024-edge supertile -> 1248 instructions/core instead of 1968+.
  - dots = reduce_d(psum * dst_rows) on DVE; host inverse-permutes.

Edge slotting: block runs are padded to C=12 chunks of 128 so the
chunk->block map is static (SPMD: one program for all cores).
"""

import sys

import numpy as np

_TRN_REPO = "/opt/trn_rl_repo"
if _TRN_REPO not in sys.path:
    sys.path.insert(0, _TRN_REPO)

import ml_dtypes

N_NODES = 100000
N_EDGES = 1000000
D = 64
N_CORES = 8
E_CORE = N_EDGES // N_CORES   # 125000

B_CORE = 101                  # local table blocks per core (seed-0 max 99)
T_ROWS = B_CORE * 128         # 12928 table rows
C = 11                        # chunk slots per block (seed-0 max run 1385)
SUPER = 16                    # chunks per supertile
CHUNKS = -(-(B_CORE * C) // SUPER) * SUPER   # 1120 (rounded to supertiles)
ST = CHUNKS // SUPER          # 70
E_PAD = CHUNKS * 128          # 143360 edge slots per core
N_SWDGE_Q = 4                 # parallel SWDGE dynamic queues

_PROGRAM_CACHE = {}


def _build_program():
    import concourse.bass as bass
    import concourse.tile as tile
    from concourse import bacc, mybir

    nc = bacc.Bacc(
        "TRN2",
        target_bir_lowering=False,
        debug=False,
        num_swdge_queues=N_SWDGE_Q,
        dynamic_dma_scratch_size=65536,
    )

    h_t = nc.dram_tensor("h_nodes", [N_NODES, D], mybir.dt.bfloat16, kind="ExternalInput")
    tab_t = nc.dram_tensor("tab", [T_ROWS, D], mybir.dt.float32, kind="ExternalInput")
    srcl_t = nc.dram_tensor("srcl", [ST, SUPER * 128], mybir.dt.bfloat16, kind="ExternalInput")
    iota_t = nc.dram_tensor("iota", [128, SUPER * 128], mybir.dt.bfloat16, kind="ExternalInput")
    di_t = nc.dram_tensor("dst_idx", [128, ST * SUPER], mybir.dt.int32, kind="ExternalInput")
    out_t = nc.dram_tensor("edot", [ST, 128, SUPER], mybir.dt.float32, kind="ExternalOutput")

    h_ap = h_t.ap()
    EW = SUPER * 128   # 1024 edges per supertile

    with tile.TileContext(nc) as tc:
        with (
            tc.tile_pool(name="tabst", bufs=1) as tabst_pool,
            tc.tile_pool(name="tab", bufs=1) as tab_pool,
        ):
            # table: [T_ROWS, D] f32 -> SBUF [128, B_CORE, D] -> bf16
            tab_f32 = tabst_pool.tile([128, B_CORE * D], mybir.dt.float32, tag="tabf")
            nc.sync.dma_start(
                out=tab_f32[:].rearrange("p (b d) -> p b d", d=D),
                in_=tab_t.ap().rearrange("(b p) d -> p b d", p=128),
            )
            tab_bf = tab_pool.tile([128, B_CORE * D], mybir.dt.bfloat16, tag="tabb")
            nc.vector.tensor_copy(out=tab_bf[:], in_=tab_f32[:])

            iota = tab_pool.tile([128, EW], mybir.dt.bfloat16, tag="iota")
            nc.sync.dma_start(out=iota[:], in_=iota_t.ap())

            # all dst indices up-front: one DMA, kills per-supertile stalls
            di_all = tab_pool.tile([128, ST * SUPER], mybir.dt.int32, tag="diall")
            nc.sync.dma_start(out=di_all[:], in_=di_t.ap())

            with (
                tc.tile_pool(name="oh", bufs=4) as oh_pool,
                tc.tile_pool(name="gat", bufs=4) as gat_pool,
                tc.tile_pool(name="res", bufs=3) as res_pool,
                tc.tile_pool(name="ps", bufs=3, space="PSUM") as ps_pool,
            ):
                for st in range(ST):
                    bc = oh_pool.tile([128, EW], mybir.dt.bfloat16, tag="bc")
                    nc.sync.dma_start(
                        out=bc[:],
                        in_=srcl_t.ap()[st][None, :].broadcast_to([128, EW]),
                    )
                    onehot = oh_pool.tile([128, EW], mybir.dt.bfloat16, tag="oh")
                    nc.vector.tensor_tensor(
                        out=onehot[:], in0=bc[:], in1=iota[:],
                        op=mybir.AluOpType.is_equal,
                    )

                    hd = gat_pool.tile([128, SUPER * D], mybir.dt.bfloat16, tag="hd")
                    for g in range(SUPER):
                        gi = nc.gpsimd.indirect_dma_start(
                            out=hd[:, g * D:(g + 1) * D],
                            out_offset=None,
                            in_=h_ap,
                            in_offset=bass.IndirectOffsetOnAxis(
                                ap=di_all[:, st * SUPER + g:st * SUPER + g + 1],
                                axis=0,
                            ),
                        )
                        q = g % N_SWDGE_Q
                        gi.ins.queue = f"qPoolDynamic{q or ''}"

                    psum = ps_pool.tile([128, SUPER * D], mybir.dt.float32, tag="ps")
                    for g in range(SUPER):
                        blk = min((st * SUPER + g) // C, B_CORE - 1)
                        nc.tensor.matmul(
                            psum[:, g * D:(g + 1) * D],
                            onehot[:, g * 128:(g + 1) * 128],
                            tab_bf[:, blk * D:(blk + 1) * D],
                            start=True, stop=True,
                        )

                    prod = gat_pool.tile([128, SUPER * D], mybir.dt.float32, tag="prod")
                    nc.vector.tensor_mul(out=prod[:], in0=psum[:], in1=hd[:])

                    dots = res_pool.tile([128, SUPER], mybir.dt.float32, tag="dots")
                    nc.vector.tensor_reduce(
                        out=dots[:],
                        in_=prod[:].rearrange("p (g d) -> p g d", d=D),
                        axis=mybir.AxisListType.X,
                        op=mybir.AluOpType.add,
                    )
                    nc.sync.dma_start(out=out_t.ap()[st], in_=dots[:])

    nc.compile()
    return nc


def _get_program():
    if "p" not in _PROGRAM_CACHE:
        _PROGRAM_CACHE["p"] = _build_program()
    return _PROGRAM_CACHE["p"]


def _prep_core(h, src_s, dst_s, eid_s):
    """Slot one core's src-sorted edge slice into the padded chunk layout.

    Returns the per-core input map plus eid_slot for host reassembly.
    """
    base = int(src_s[0]) // 128 * 128
    local = src_s.astype(np.int64) - base
    blk = local >> 7
    maxblk = int(blk[-1])
    if maxblk >= B_CORE:
        raise RuntimeError(f"core spans {maxblk + 1} blocks > B_CORE={B_CORE}")
    counts = np.bincount(blk, minlength=B_CORE)
    if counts.max() > C * 128:
        raise RuntimeError(f"block run {counts.max()} > {C * 128}")
    starts = np.concatenate([[0], np.cumsum(counts)[:-1]])
    rank = np.arange(len(src_s)) - starts[blk]
    slot = blk * (C * 128) + rank          # [E_CORE] slot ids in [0, E_PAD)

    srclocal = np.zeros(E_PAD, dtype=np.float32)
    dst_slot = np.zeros(E_PAD, dtype=np.int32)
    eid_slot = np.full(E_PAD, -1, dtype=np.int64)
    srclocal[slot] = (local & 127).astype(np.float32)
    dst_slot[slot] = dst_s.astype(np.int32)
    eid_slot[slot] = eid_s

    # table slice (zero-padded past N_NODES)
    tab = np.zeros((T_ROWS, D), dtype=np.float32)
    hi = min(base + T_ROWS, N_NODES)
    tab[: hi - base] = h[base:hi]

    # device layouts
    srcl = np.ascontiguousarray(
        srclocal.reshape(ST, SUPER * 128).astype(ml_dtypes.bfloat16)
    )
    # slot (k, col p) -> dst_idx[p, k]  (one contiguous [128, CHUNKS] DMA)
    dst_in = np.ascontiguousarray(dst_slot.reshape(CHUNKS, 128).T)
    return (
        {"tab": tab, "srcl": srcl, "dst_idx": dst_in},
        eid_slot,
    )


def _run(h, src, dst, trace=False):
    from concourse.bass_utils import run_bass_kernel_spmd

    h = np.ascontiguousarray(np.asarray(h, dtype=np.float32))
    src = np.asarray(src).astype(np.int64)
    dst = np.asarray(dst).astype(np.int64)

    h_bf = h.astype(ml_dtypes.bfloat16)
    perm = np.argsort(src, kind="stable")
    src_s = src[perm]
    dst_s = dst[perm]

    iota = np.ascontiguousarray(
        np.tile(
            np.arange(128, dtype=np.float32)[:, None], (1, SUPER * 128)
        ).astype(ml_dtypes.bfloat16)
    )

    in_maps = []
    eid_slots = []
    for c in range(N_CORES):
        sl = slice(c * E_CORE, (c + 1) * E_CORE)
        m, eid_slot = _prep_core(h, src_s[sl], dst_s[sl], perm[sl])
        m["h_nodes"] = h_bf
        m["iota"] = iota
        in_maps.append(m)
        eid_slots.append(eid_slot)

    nc = _get_program()
    res = run_bass_kernel_spmd(nc, in_maps, list(range(N_CORES)), trace=trace)

    out = np.empty(N_EDGES, dtype=np.float32)
    for c in range(N_CORES):
        dots = np.asarray(res.results[c]["edot"])   # [ST, 128, SUPER]
        flat = dots.transpose(0, 2, 1).reshape(E_PAD)  # slot order
        eid_slot = eid_slots[c]
        valid = eid_slot >= 0
        out[eid_slot[valid]] = flat[valid]
    return out, res


def kernel(h, src, dst):
    out, _ = _run(h, src, dst)
    return out


# revision 22
# speedup vs baseline: 1.0840x; 1.0840x over previous
"""Per-edge dot product kernel for Trainium2 (8 NeuronCores).

Computes out[e] = sum(h[src[e]] * h[dst[e]], axis=-1) for
h: [100000, 64] f32, src/dst: [1000000] int indices.

Bottleneck analysis (HW, bedrock image -- no extended GPSIMD ucode, so
dma_gather/ap_gather are unavailable and the only data-dependent gather
is core SWDGE indirect DMA):
  - indirect_dma_start costs ~994ns fixed SWDGE overhead + 0.34ns/desc,
    and HW supports only ONE offset per partition -> 128 rows / ~1.1us
    instruction, serialized on the GPSIMD engine.  The old kernel issued
    2*E/8/128 = 1968 such instructions per core -> 2.2ms GPSIMD time.

Design (halves the indirect-DMA count by moving the src side to PE):
  - Host sorts edges by src; core c takes the c-th contiguous 125k slice
    of sorted order, so its src values span ~12.6k contiguous node rows.
    That slice of h is the core's "table", DMA'd sequentially (no
    descriptors-per-row) and held in SBUF as bf16 [128, 104 blocks, 64].
  - Src rows are then gathered ON-CHIP via one-hot matmuls: for each
    chunk of 128 edges (all in one 128-row table block by construction),
    PE computes onehot[128nodes,128edges]^T @ table_blk[128,64] ->
    gathered rows [128 edges, 64] in PSUM.  One-hots are built by DVE
    is_equal(broadcast(srclocal), iota) -- no GPSIMD involvement.
  - Dst rows (random order) still use indirect DMA: 8 gathers per
    1024-edge supertile -> 1248 instructions/core instead of 1968+.
  - dots = reduce_d(psum * dst_rows) on DVE; host inverse-permutes.

Edge slotting: block runs are padded to C=12 chunks of 128 so the
chunk->block map is static (SPMD: one program for all cores).
"""

import sys

import numpy as np

_TRN_REPO = "/opt/trn_rl_repo"
if _TRN_REPO not in sys.path:
    sys.path.insert(0, _TRN_REPO)

import ml_dtypes

N_NODES = 100000
N_EDGES = 1000000
D = 64
N_CORES = 8
E_CORE = N_EDGES // N_CORES   # 125000

B_CORE = 100                  # local table blocks per core (seed-0 max 99)
T_ROWS = B_CORE * 128         # 12800 table rows
C = 10                        # chunk slots per block; excess edges overflow
SUPER = 8                     # chunks per supertile
CHUNKS = B_CORE * C           # 1000
ST = CHUNKS // SUPER          # 125
E_PAD = CHUNKS * 128          # 128000 main edge slots per core
OVF_CHUNKS = 16               # both-sides-indirect overflow chunks
OVF_PAD = OVF_CHUNKS * 128    # 2048 overflow slots
N_SWDGE_Q = 4                 # parallel SWDGE dynamic queues

_PROGRAM_CACHE = {}


def _build_program():
    import concourse.bass as bass
    import concourse.tile as tile
    from concourse import bacc, mybir

    nc = bacc.Bacc(
        "TRN2",
        target_bir_lowering=False,
        debug=False,
        num_swdge_queues=N_SWDGE_Q,
        dynamic_dma_scratch_size=65536,
    )

    h_t = nc.dram_tensor("h_nodes", [N_NODES, D], mybir.dt.bfloat16, kind="ExternalInput")
    tab_t = nc.dram_tensor("tab", [T_ROWS, D], mybir.dt.float32, kind="ExternalInput")
    srcl_t = nc.dram_tensor("srcl", [ST, SUPER * 128], mybir.dt.bfloat16, kind="ExternalInput")
    iota_t = nc.dram_tensor("iota", [128, SUPER * 128], mybir.dt.bfloat16, kind="ExternalInput")
    di_t = nc.dram_tensor("dst_idx", [128, ST * SUPER], mybir.dt.int32, kind="ExternalInput")
    ovf_si_t = nc.dram_tensor("ovf_src", [128, OVF_CHUNKS], mybir.dt.int32, kind="ExternalInput")
    ovf_di_t = nc.dram_tensor("ovf_dst", [128, OVF_CHUNKS], mybir.dt.int32, kind="ExternalInput")
    out_t = nc.dram_tensor("edot", [ST, 128, SUPER], mybir.dt.float32, kind="ExternalOutput")
    ovf_out_t = nc.dram_tensor("edot_ovf", [128, OVF_CHUNKS], mybir.dt.float32, kind="ExternalOutput")

    h_ap = h_t.ap()
    EW = SUPER * 128   # 1024 edges per supertile

    with tile.TileContext(nc) as tc:
        with (
            tc.tile_pool(name="tabst", bufs=1) as tabst_pool,
            tc.tile_pool(name="tab", bufs=1) as tab_pool,
        ):
            # table: [T_ROWS, D] f32 -> SBUF [128, B_CORE, D] -> bf16
            tab_f32 = tabst_pool.tile([128, B_CORE * D], mybir.dt.float32, tag="tabf")
            nc.sync.dma_start(
                out=tab_f32[:].rearrange("p (b d) -> p b d", d=D),
                in_=tab_t.ap().rearrange("(b p) d -> p b d", p=128),
            )
            tab_bf = tab_pool.tile([128, B_CORE * D], mybir.dt.bfloat16, tag="tabb")
            nc.vector.tensor_copy(out=tab_bf[:], in_=tab_f32[:])

            iota = tab_pool.tile([128, EW], mybir.dt.bfloat16, tag="iota")
            nc.sync.dma_start(out=iota[:], in_=iota_t.ap())

            # all dst indices up-front: one DMA, kills per-supertile stalls
            di_all = tab_pool.tile([128, ST * SUPER], mybir.dt.int32, tag="diall")
            nc.sync.dma_start(out=di_all[:], in_=di_t.ap())

            # overflow edges (block runs > C*128): both sides via indirect
            ovf_si = tab_pool.tile([128, OVF_CHUNKS], mybir.dt.int32, tag="ovfsi")
            nc.sync.dma_start(out=ovf_si[:], in_=ovf_si_t.ap())
            ovf_di = tab_pool.tile([128, OVF_CHUNKS], mybir.dt.int32, tag="ovfdi")
            nc.sync.dma_start(out=ovf_di[:], in_=ovf_di_t.ap())
            ovf_s = tab_pool.tile([128, OVF_CHUNKS * D], mybir.dt.bfloat16, tag="ovfs")
            ovf_d = tab_pool.tile([128, OVF_CHUNKS * D], mybir.dt.bfloat16, tag="ovfd")
            for j in range(OVF_CHUNKS):
                for buf, idx in ((ovf_s, ovf_si), (ovf_d, ovf_di)):
                    gi = nc.gpsimd.indirect_dma_start(
                        out=buf[:, j * D:(j + 1) * D],
                        out_offset=None,
                        in_=h_t.ap(),
                        in_offset=bass.IndirectOffsetOnAxis(
                            ap=idx[:, j:j + 1], axis=0
                        ),
                    )
                    gi.ins.queue = f"qPoolDynamic{(j % N_SWDGE_Q) or ''}"
            ovf_p = tab_pool.tile([128, OVF_CHUNKS * D], mybir.dt.float32, tag="ovfp")
            nc.vector.tensor_mul(out=ovf_p[:], in0=ovf_s[:], in1=ovf_d[:])
            ovf_dots = tab_pool.tile([128, OVF_CHUNKS], mybir.dt.float32, tag="ovfo")
            nc.vector.tensor_reduce(
                out=ovf_dots[:],
                in_=ovf_p[:].rearrange("p (c d) -> p c d", d=D),
                axis=mybir.AxisListType.X,
                op=mybir.AluOpType.add,
            )
            nc.sync.dma_start(out=ovf_out_t.ap(), in_=ovf_dots[:])

            with (
                tc.tile_pool(name="oh", bufs=4) as oh_pool,
                tc.tile_pool(name="gat", bufs=4) as gat_pool,
                tc.tile_pool(name="res", bufs=3) as res_pool,
                tc.tile_pool(name="ps", bufs=3, space="PSUM") as ps_pool,
            ):
                for st in range(ST):
                    bc = oh_pool.tile([128, EW], mybir.dt.bfloat16, tag="bc")
                    nc.sync.dma_start(
                        out=bc[:],
                        in_=srcl_t.ap()[st][None, :].broadcast_to([128, EW]),
                    )
                    onehot = oh_pool.tile([128, EW], mybir.dt.bfloat16, tag="oh")
                    nc.vector.tensor_tensor(
                        out=onehot[:], in0=bc[:], in1=iota[:],
                        op=mybir.AluOpType.is_equal,
                    )

                    hd = gat_pool.tile([128, SUPER * D], mybir.dt.bfloat16, tag="hd")
                    for g in range(SUPER):
                        gi = nc.gpsimd.indirect_dma_start(
                            out=hd[:, g * D:(g + 1) * D],
                            out_offset=None,
                            in_=h_ap,
                            in_offset=bass.IndirectOffsetOnAxis(
                                ap=di_all[:, st * SUPER + g:st * SUPER + g + 1],
                                axis=0,
                            ),
                        )
                        q = g % N_SWDGE_Q
                        gi.ins.queue = f"qPoolDynamic{q or ''}"

                    psum = ps_pool.tile([128, SUPER * D], mybir.dt.float32, tag="ps")
                    for g in range(SUPER):
                        blk = min((st * SUPER + g) // C, B_CORE - 1)
                        nc.tensor.matmul(
                            psum[:, g * D:(g + 1) * D],
                            onehot[:, g * 128:(g + 1) * 128],
                            tab_bf[:, blk * D:(blk + 1) * D],
                            start=True, stop=True,
                        )

                    prod = gat_pool.tile([128, SUPER * D], mybir.dt.float32, tag="prod")
                    nc.vector.tensor_mul(out=prod[:], in0=psum[:], in1=hd[:])

                    dots = res_pool.tile([128, SUPER], mybir.dt.float32, tag="dots")
                    nc.vector.tensor_reduce(
                        out=dots[:],
                        in_=prod[:].rearrange("p (g d) -> p g d", d=D),
                        axis=mybir.AxisListType.X,
                        op=mybir.AluOpType.add,
                    )
                    nc.sync.dma_start(out=out_t.ap()[st], in_=dots[:])

    nc.compile()
    return nc


def _get_program():
    if "p" not in _PROGRAM_CACHE:
        _PROGRAM_CACHE["p"] = _build_program()
    return _PROGRAM_CACHE["p"]


def _prep_core(h, src_s, dst_s, eid_s):
    """Slot one core's src-sorted edge slice into the padded chunk layout.

    Returns the per-core input map plus eid_slot for host reassembly.
    """
    base = int(src_s[0]) // 128 * 128
    local = src_s.astype(np.int64) - base
    blk = local >> 7
    maxblk = int(blk[-1])
    if maxblk >= B_CORE:
        raise RuntimeError(f"core spans {maxblk + 1} blocks > B_CORE={B_CORE}")
    counts = np.bincount(blk, minlength=B_CORE)
    starts = np.concatenate([[0], np.cumsum(counts)[:-1]])
    rank = np.arange(len(src_s)) - starts[blk]

    # main slots: first C*128 edges of each block run
    main = rank < C * 128
    ovf = ~main
    n_ovf = int(ovf.sum())
    if n_ovf > OVF_PAD:
        raise RuntimeError(f"overflow {n_ovf} > {OVF_PAD}")
    slot = blk[main] * (C * 128) + rank[main]

    srclocal = np.zeros(E_PAD, dtype=np.float32)
    dst_slot = np.zeros(E_PAD, dtype=np.int32)
    eid_slot = np.full(E_PAD, -1, dtype=np.int64)
    srclocal[slot] = (local[main] & 127).astype(np.float32)
    dst_slot[slot] = dst_s[main].astype(np.int32)
    eid_slot[slot] = eid_s[main]

    ovf_src = np.zeros(OVF_PAD, dtype=np.int32)
    ovf_dst = np.zeros(OVF_PAD, dtype=np.int32)
    ovf_eid = np.full(OVF_PAD, -1, dtype=np.int64)
    ovf_src[:n_ovf] = src_s[ovf].astype(np.int32)
    ovf_dst[:n_ovf] = dst_s[ovf].astype(np.int32)
    ovf_eid[:n_ovf] = eid_s[ovf]

    # table slice (zero-padded past N_NODES)
    tab = np.zeros((T_ROWS, D), dtype=np.float32)
    hi = min(base + T_ROWS, N_NODES)
    tab[: hi - base] = h[base:hi]

    # device layouts
    srcl = np.ascontiguousarray(
        srclocal.reshape(ST, SUPER * 128).astype(ml_dtypes.bfloat16)
    )
    # slot (k, col p) -> dst_idx[p, k]  (one contiguous [128, CHUNKS] DMA)
    dst_in = np.ascontiguousarray(dst_slot.reshape(CHUNKS, 128).T)
    # overflow slot (j, col p) -> [p, j]
    ovf_src_in = np.ascontiguousarray(ovf_src.reshape(OVF_CHUNKS, 128).T)
    ovf_dst_in = np.ascontiguousarray(ovf_dst.reshape(OVF_CHUNKS, 128).T)
    return (
        {
            "tab": tab,
            "srcl": srcl,
            "dst_idx": dst_in,
            "ovf_src": ovf_src_in,
            "ovf_dst": ovf_dst_in,
        },
        eid_slot,
        ovf_eid,
    )


def _run(h, src, dst, trace=False):
    from concourse.bass_utils import run_bass_kernel_spmd

    h = np.ascontiguousarray(np.asarray(h, dtype=np.float32))
    src = np.asarray(src).astype(np.int64)
    dst = np.asarray(dst).astype(np.int64)

    h_bf = h.astype(ml_dtypes.bfloat16)
    perm = np.argsort(src, kind="stable")
    src_s = src[perm]
    dst_s = dst[perm]

    iota = np.ascontiguousarray(
        np.tile(
            np.arange(128, dtype=np.float32)[:, None], (1, SUPER * 128)
        ).astype(ml_dtypes.bfloat16)
    )

    in_maps = []
    eid_slots = []
    ovf_eids = []
    for c in range(N_CORES):
        sl = slice(c * E_CORE, (c + 1) * E_CORE)
        m, eid_slot, ovf_eid = _prep_core(h, src_s[sl], dst_s[sl], perm[sl])
        m["h_nodes"] = h_bf
        m["iota"] = iota
        in_maps.append(m)
        eid_slots.append(eid_slot)
        ovf_eids.append(ovf_eid)

    nc = _get_program()
    res = run_bass_kernel_spmd(nc, in_maps, list(range(N_CORES)), trace=trace)

    out = np.empty(N_EDGES, dtype=np.float32)
    for c in range(N_CORES):
        dots = np.asarray(res.results[c]["edot"])   # [ST, 128, SUPER]
        flat = dots.transpose(0, 2, 1).reshape(E_PAD)  # slot order
        eid_slot = eid_slots[c]
        valid = eid_slot >= 0
        out[eid_slot[valid]] = flat[valid]

        odots = np.asarray(res.results[c]["edot_ovf"])  # [128, OVF_CHUNKS]
        oflat = odots.T.reshape(OVF_PAD)                # slot (j, p) order
        ovf_eid = ovf_eids[c]
        ovalid = ovf_eid >= 0
        out[ovf_eid[ovalid]] = oflat[ovalid]
    return out, res


def kernel(h, src, dst):
    out, _ = _run(h, src, dst)
    return out
